# revision 10
# baseline (speedup 1.0000x reference)
"""Trainium2 Bass kernel for nn_LinearAttention (3D linear-attention block).

Math (faithful to the reference, but algebraically restructured):
  qkv = w_qkv @ x              (pointwise conv, x: [256, 32^3])
  k,v are trilinearly downsampled to 8^3 BEFORE the projection
      (downsample commutes with the channel projection -> huge FLOP saving:
       k = W_k @ (x @ M3^T) instead of downsample(W_k @ x))
  per head h (8 heads, dim_head 64, channel c = d*8 + h):
      attn = softmax(q_h^T k_h * 64^-0.5)   [N=32768, M=512]
      o_h = attn @ v_h
  out = w_out @ o

Sharding: split the depth axis D=32 into 8 slices of 4 planes -> each core
handles 4096 query positions (all heads). k/v (512 keys) are computed
redundantly on every core from a small replicated subset of x: the trilinear
interp with align_corners=True only reads 16 of 32 planes per axis, so the
needed input is x[:, idx16, idx16, idx16] = [256, 4096] (4 MB).

Per-core device program:
  x_red[c, key]   = sum_n xneedT[n, c] * M3cT[n, key]     (PE, K=4096 accum)
  kv[oc, key]     = Wkv head-major proj of x_red          (PE)
  vT per head     = PE transpose of v_h                   (PE)
  q[m, pos]       = Wq head-major proj of x slice         (PE)
  per (head, pos-chunk of 128):
      logits (PSUM) -> exp(scale*logits) on ScalarE with accumulated row sums
      -> reciprocal + row-normalize on VectorE -> DMA attn out
      -> PE-transpose of normalized attn -> o = attnT-matmul with vT (PE)
  out = Wout-proj of o                                    (PE)
"""

import numpy as np

HEADS = 8
DIM_HEAD = 64
INNER = 512
DIM = 256
R = 8
SCALE = DIM_HEAD ** -0.5
D = H = W = 32
N = D * H * W          # 32768
NCORES = 8
DSL = D // NCORES      # 4 depth planes per core
NPOS = DSL * H * W     # 4096 query positions per core
NKEY = R * R * R       # 512 keys
NEED = 16              # input grid points read by the 32->8 interp, per axis

_F32 = np.float32


def _interp_matrix(n_in, n_out):
    # replicate reference.interp_matrix (f32, align_corners=True)
    pos = np.arange(n_out, dtype=_F32) * _F32((n_in - 1) / (n_out - 1))
    lo = np.clip(np.floor(pos).astype(np.int32), 0, n_in - 2)
    frac = pos - lo.astype(_F32)
    M = np.zeros((n_out, n_in), _F32)
    rows = np.arange(n_out)
    M[rows, lo] += _F32(1.0) - frac
    M[rows, lo + 1] += frac
    return M, lo


def _host_constants():
    M, lo = _interp_matrix(D, R)
    idx = np.unique(np.concatenate([lo, lo + 1]))          # 16 needed planes
    assert len(idx) == NEED
    Mc = np.ascontiguousarray(M[:, idx])                   # [8, 16]
    # compact 3D interp operator [512 keys, 4096 needed points]
    M3c = np.einsum("ai,bj,ck->abcijk", Mc, Mc, Mc).reshape(NKEY, NEED ** 3)
    M3cT = np.ascontiguousarray(M3c.T.astype(_F32))        # [4096, 512]
    # head-major channel permutation: hm[h*64+d] = d*8+h
    hm = (np.arange(HEADS)[:, None] + HEADS * np.arange(DIM_HEAD)[None, :]).reshape(-1)
    return idx, M3cT, hm


_NC_CACHE = {}


def _build_nc():
    import concourse.bacc as bacc
    import concourse.tile as tile
    from concourse import mybir

    f32 = mybir.dt.float32
    Exp = mybir.ActivationFunctionType.Exp

    nc = bacc.Bacc("TRN2", target_bir_lowering=False, debug=False)

    xs_d = nc.dram_tensor("xs", [DIM, NPOS], f32, kind="ExternalInput")
    xneedT_d = nc.dram_tensor("xneedT", [NEED ** 3, DIM], f32, kind="ExternalInput")
    m3cT_d = nc.dram_tensor("m3cT", [NEED ** 3, NKEY], f32, kind="ExternalInput")
    wqT_d = nc.dram_tensor("wqT", [DIM, INNER], f32, kind="ExternalInput")
    wkvT_d = nc.dram_tensor("wkvT", [DIM, 2 * INNER], f32, kind="ExternalInput")
    woT_d = nc.dram_tensor("woT", [INNER, DIM], f32, kind="ExternalInput")
    ident_d = nc.dram_tensor("ident", [128, 128], f32, kind="ExternalInput")

    out_d = nc.dram_tensor("out_s", [DIM, NPOS], f32, kind="ExternalOutput")
    attn_d = nc.dram_tensor("attn_s", [HEADS, NPOS, NKEY], f32, kind="ExternalOutput")

    out_r = out_d[:, :].rearrange("(mc p) n -> p mc n", p=128)   # [128, 2, 4096]

    with tile.TileContext(nc) as tc:
        with (
            tc.tile_pool(name="const", bufs=1) as constp,
            tc.tile_pool(name="stream", bufs=4) as streamp,
            tc.tile_pool(name="work", bufs=3) as workp,
            tc.tile_pool(name="expT", bufs=2) as expTp,
            tc.tile_pool(name="qg", bufs=2) as qgp,
            tc.tile_pool(name="og", bufs=2) as ogp,
            tc.tile_pool(name="small", bufs=8) as smallp,
            tc.tile_pool(name="psA", bufs=4, space="PSUM") as psmm,
            tc.tile_pool(name="psB", bufs=4, space="PSUM") as pstr,
        ):
            ident_sb = constp.tile([128, 128], f32, tag="ident")
            nc.sync.dma_start(ident_sb, ident_d[:, :])
            wq_sb = constp.tile([128, 2, INNER], f32, tag="wq")
            nc.sync.dma_start(wq_sb, wqT_d[:, :].rearrange("(c p) m -> p c m", p=128))
            wkv_sb = constp.tile([128, 2, 2 * INNER], f32, tag="wkv")
            nc.sync.dma_start(wkv_sb, wkvT_d[:, :].rearrange("(c p) m -> p c m", p=128))
            wo_sb = constp.tile([128, 4, DIM], f32, tag="wo")
            nc.sync.dma_start(wo_sb, woT_d[:, :].rearrange("(c p) m -> p c m", p=128))
            x_sb = constp.tile([128, 2, NPOS], f32, tag="x")
            nc.sync.dma_start(x_sb, xs_d[:, :].rearrange("(c p) n -> p c n", p=128))

            xred_sb = constp.tile([128, 2, NKEY], f32, tag="xred")
            kv_sb = constp.tile([128, 8, NKEY], f32, tag="kv")
            vT_sb = constp.tile([128, HEADS, 4, DIM_HEAD], f32, tag="vT")

            # ---- stage A: x_red[c, key] (two 128-channel chunks, K=4096) ----
            xn_sb = constp.tile([128, 32, DIM], f32, tag="xn")
            nc.sync.dma_start(
                xn_sb, xneedT_d[:, :].rearrange("(kc p) c -> p kc c", p=128)
            )
            psA = [
                psmm.tile([128, NKEY], f32, tag="mm", name=f"psA{_mc}")
                for _mc in range(2)
            ]
            for kc in range(32):
                m3_t = streamp.tile([128, NKEY], f32, tag="m3")
                nc.sync.dma_start(m3_t, m3cT_d[kc * 128:(kc + 1) * 128, :])
                for mc in range(2):
                    nc.tensor.matmul(
                        psA[mc], xn_sb[:, kc, mc * 128:(mc + 1) * 128], m3_t,
                        start=(kc == 0), stop=(kc == 31), skip_group_check=True,
                    )
            for mc in range(2):
                nc.vector.tensor_copy(xred_sb[:, mc, :], psA[mc])

            # ---- stage B: k,v projections (head-major channels) ----
            for oc in range(8):
                ps = psmm.tile([128, NKEY], f32, tag="mm")
                for cc in range(2):
                    nc.tensor.matmul(
                        ps, wkv_sb[:, cc, oc * 128:(oc + 1) * 128], xred_sb[:, cc, :],
                        start=(cc == 0), stop=(cc == 1),
                    )
                nc.vector.tensor_copy(kv_sb[:, oc, :], ps)

            # ---- stage C: vT per head (PE transposes of v_h) ----
            for h in range(HEADS):
                p0 = (h % 2) * 64
                for jc in range(4):
                    pst = pstr.tile([128, 128], f32, tag="tr")
                    nc.tensor.transpose(
                        pst[:, :64],
                        kv_sb[p0:p0 + 64, 4 + h // 2, jc * 128:(jc + 1) * 128],
                        ident_sb[p0:p0 + 64, p0:p0 + 64],
                    )
                    nc.vector.tensor_copy(vT_sb[:, h, jc, :], pst[:, :64])

            # ---- main loop: q projection + attention, per 512-position group ----
            for g in range(8):
                q_g = qgp.tile([128, 4, 512], f32, tag="qg")
                for mc in range(4):
                    ps = psmm.tile([128, 512], f32, tag="mm")
                    for cc in range(2):
                        nc.tensor.matmul(
                            ps, wq_sb[:, cc, mc * 128:(mc + 1) * 128],
                            x_sb[:, cc, g * 512:(g + 1) * 512],
                            start=(cc == 0), stop=(cc == 1),
                        )
                    if mc % 2 == 0:
                        nc.scalar.copy(q_g[:, mc, :], ps)
                    else:
                        nc.vector.tensor_copy(q_g[:, mc, :], ps)

                o_g = ogp.tile([128, 4, 512], f32, tag="og")
                for h in range(HEADS):
                    p0 = (h % 2) * 64
                    mcq = h // 2
                    expT_t = expTp.tile([128, 4, 512], f32, tag="expT")
                    for pc in range(4):
                        c = g * 4 + pc
                        ps_a = psmm.tile([128, NKEY], f32, tag="mm")
                        nc.tensor.matmul(
                            ps_a,
                            q_g[p0:p0 + 64, mcq, pc * 128:(pc + 1) * 128],
                            kv_sb[p0:p0 + 64, mcq, :],
                            start=True, stop=True,
                        )
                        exp_t = workp.tile([128, NKEY], f32, tag="exp")
                        sum_t = smallp.tile([128, 1], f32, tag="sum")
                        nc.scalar.activation(exp_t, ps_a, Exp, scale=SCALE, accum_out=sum_t)
                        rec_t = smallp.tile([128, 1], f32, tag="rec")
                        nc.vector.reciprocal(rec_t, sum_t)
                        attn_t = workp.tile([128, NKEY], f32, tag="attn")
                        nc.vector.tensor_scalar_mul(attn_t, exp_t, rec_t)
                        nc.sync.dma_start(attn_d[h, c * 128:(c + 1) * 128, :], attn_t)
                        for jc in range(4):
                            pst = pstr.tile([128, 128], f32, tag="tr")
                            nc.tensor.transpose(
                                pst, attn_t[:, jc * 128:(jc + 1) * 128], ident_sb
                            )
                            if jc % 2 == 0:
                                nc.vector.tensor_copy(
                                    expT_t[:, jc, pc * 128:(pc + 1) * 128], pst
                                )
                            else:
                                nc.scalar.copy(
                                    expT_t[:, jc, pc * 128:(pc + 1) * 128], pst
                                )
                    ps_o = psmm.tile([64, 512], f32, tag="mm")
                    for jc in range(4):
                        nc.tensor.matmul(
                            ps_o, vT_sb[:, h, jc, :], expT_t[:, jc, :],
                            start=(jc == 0), stop=(jc == 3),
                        )
                    if h % 2 == 0:
                        nc.vector.tensor_copy(o_g[p0:p0 + 64, mcq, :], ps_o)
                    else:
                        nc.scalar.copy(o_g[p0:p0 + 64, mcq, :], ps_o)

                # ---- out projection for this pos-group ----
                for mc2 in range(2):
                    ps_f = psmm.tile([128, 512], f32, tag="mm")
                    for oc in range(4):
                        nc.tensor.matmul(
                            ps_f, wo_sb[:, oc, mc2 * 128:(mc2 + 1) * 128], o_g[:, oc, :],
                            start=(oc == 0), stop=(oc == 3),
                        )
                    out_t = workp.tile([128, 512], f32, tag="outt")
                    nc.vector.tensor_copy(out_t, ps_f)
                    nc.sync.dma_start(out_r[:, mc2, g * 512:(g + 1) * 512], out_t)

    nc.finalize()
    return nc


def kernel(x, w_qkv, w_out):
    from concourse.bass_utils import run_bass_kernel_spmd

    x = np.asarray(x, dtype=_F32)
    w_qkv = np.asarray(w_qkv, dtype=_F32)
    w_out = np.asarray(w_out, dtype=_F32)

    idx, M3cT, hm = _host_constants()

    xf = x[0]                                             # [256, 32, 32, 32]
    xneed = xf[:, idx][:, :, idx][:, :, :, idx].reshape(DIM, NEED ** 3)
    xneedT = np.ascontiguousarray(xneed.T)                # [4096, 256]

    Wq = w_qkv[0:INNER][hm]                               # head-major rows
    Wk = w_qkv[INNER:2 * INNER][hm]
    Wv = w_qkv[2 * INNER:3 * INNER][hm]
    wqT = np.ascontiguousarray(Wq.T)                      # [256, 512]
    wkvT = np.ascontiguousarray(np.concatenate([Wk, Wv], 0).T)   # [256, 1024]
    woT = np.ascontiguousarray(w_out[:, hm].T)            # [512, 256]
    ident = np.eye(128, dtype=_F32)

    if "nc" not in _NC_CACHE:
        _NC_CACHE["nc"] = _build_nc()
    nc = _NC_CACHE["nc"]

    shared = {
        "xneedT": xneedT, "m3cT": M3cT, "wqT": wqT, "wkvT": wkvT,
        "woT": woT, "ident": ident,
    }
    in_maps = []
    for i in range(NCORES):
        xs = np.ascontiguousarray(
            xf[:, i * DSL:(i + 1) * DSL].reshape(DIM, NPOS)
        )
        in_maps.append({"xs": xs, **shared})

    import os
    trace = bool(int(os.environ.get("KERNEL_TRACE", "0")))
    res = run_bass_kernel_spmd(
        nc, in_maps, core_ids=list(range(NCORES)), trace=trace
    )
    _NC_CACHE["last_result"] = res
    results = res.results

    out = np.concatenate(
        [r["out_s"].reshape(DIM, DSL, H, W) for r in results], axis=1
    )[None]                                               # [1, 256, 32, 32, 32]
    attn = np.concatenate([r["attn_s"] for r in results], axis=1)[None]
    return out, attn


# revision 11
# speedup vs baseline: 1.1839x; 1.1839x over previous
"""Trainium2 Bass kernel for nn_LinearAttention (3D linear-attention block).

Math (faithful to the reference, but algebraically restructured):
  qkv = w_qkv @ x              (pointwise conv, x: [256, 32^3])
  k,v are trilinearly downsampled to 8^3 BEFORE the projection
      (downsample commutes with the channel projection -> huge FLOP saving)
  per head h (8 heads, dim_head 64, channel c = d*8 + h):
      attn = softmax(q_h^T k_h * 64^-0.5)   [N=32768, M=512]
      o_h = attn @ v_h
  out = w_out @ o

Sharding: split the depth axis D=32 into 8 slices of 4 planes -> each core
handles 4096 query positions (all heads). k/v (512 keys) are computed
redundantly on every core from a small replicated subset of x (the trilinear
interp with align_corners=True only reads 16 of 32 planes per axis).

Performance notes:
  - fp32 matmuls lower to TWO hardware passes on TRN2; float32r (rounded
    fp32, ~tf32 precision: measured 1.5e-4 rel err) runs single-pass at
    ~227ns per 512-col matmul.  All matmuls take float32r operands; the
    rounding "producer" ops walrus requires are folded into the PSUM->SBUF
    evacuation copies where possible, and done on the otherwise-idle GpSimd
    engine for DMA-fed operands.
  - exp(scale*logits) runs on ScalarE straight out of PSUM with accumulated
    row sums (accum_out), softmax normalize on VectorE.
  - o = attn @ v needs attn^T; done via PE transposes of the normalized
    attn tiles (fp32, single pass), evacuated to f32r SBUF tiles split
    between ScalarE and VectorE.
"""

import numpy as np

HEADS = 8
DIM_HEAD = 64
INNER = 512
DIM = 256
R = 8
SCALE = DIM_HEAD ** -0.5
D = H = W = 32
N = D * H * W          # 32768
NCORES = 8
DSL = D // NCORES      # 4 depth planes per core
NPOS = DSL * H * W     # 4096 query positions per core
NKEY = R * R * R       # 512 keys
NEED = 16              # input grid points read by the 32->8 interp, per axis

_F32 = np.float32


def _interp_matrix(n_in, n_out):
    # replicate reference.interp_matrix (f32, align_corners=True)
    pos = np.arange(n_out, dtype=_F32) * _F32((n_in - 1) / (n_out - 1))
    lo = np.clip(np.floor(pos).astype(np.int32), 0, n_in - 2)
    frac = pos - lo.astype(_F32)
    M = np.zeros((n_out, n_in), _F32)
    rows = np.arange(n_out)
    M[rows, lo] += _F32(1.0) - frac
    M[rows, lo + 1] += frac
    return M, lo


def _host_constants():
    M, lo = _interp_matrix(D, R)
    idx = np.unique(np.concatenate([lo, lo + 1]))          # 16 needed planes
    assert len(idx) == NEED
    Mc = np.ascontiguousarray(M[:, idx])                   # [8, 16]
    # compact 3D interp operator [512 keys, 4096 needed points]
    M3c = np.einsum("ai,bj,ck->abcijk", Mc, Mc, Mc).reshape(NKEY, NEED ** 3)
    M3cT = np.ascontiguousarray(M3c.T.astype(_F32))        # [4096, 512]
    # head-major channel permutation: hm[h*64+d] = d*8+h
    hm = (np.arange(HEADS)[:, None] + HEADS * np.arange(DIM_HEAD)[None, :]).reshape(-1)
    return idx, M3cT, hm


_NC_CACHE = {}


def _build_nc():
    import concourse.bacc as bacc
    import concourse.tile as tile
    from concourse import mybir

    f32 = mybir.dt.float32
    f32r = mybir.dt.float32r
    Exp = mybir.ActivationFunctionType.Exp

    nc = bacc.Bacc("TRN2", target_bir_lowering=False, debug=False)

    xs_d = nc.dram_tensor("xs", [DIM, NPOS], f32, kind="ExternalInput")
    xneedT_d = nc.dram_tensor("xneedT", [NEED ** 3, DIM], f32, kind="ExternalInput")
    m3cT_d = nc.dram_tensor("m3cT", [NEED ** 3, NKEY], f32, kind="ExternalInput")
    wqT_d = nc.dram_tensor("wqT", [DIM, INNER], f32, kind="ExternalInput")
    wkvT_d = nc.dram_tensor("wkvT", [DIM, 2 * INNER], f32, kind="ExternalInput")
    woT_d = nc.dram_tensor("woT", [INNER, DIM], f32, kind="ExternalInput")
    ident_d = nc.dram_tensor("ident", [128, 128], f32, kind="ExternalInput")

    out_d = nc.dram_tensor("out_s", [DIM, NPOS], f32, kind="ExternalOutput")
    attn_d = nc.dram_tensor("attn_s", [HEADS, NPOS, NKEY], f32, kind="ExternalOutput")

    out_r = out_d[:, :].rearrange("(mc p) n -> p mc n", p=128)   # [128, 2, 4096]
    xs_r = xs_d[:, :].rearrange("(c p) n -> p c n", p=128)       # [128, 2, 4096]
    wq_r_d = wqT_d[:, :].rearrange("(c p) m -> p c m", p=128)    # [128, 2, 512]
    wkv_r_d = wkvT_d[:, :].rearrange("(c p) m -> p c m", p=128)  # [128, 2, 1024]
    wo_r_d = woT_d[:, :].rearrange("(c p) m -> p c m", p=128)    # [128, 4, 256]
    xn_r_d = xneedT_d[:, :].rearrange("(kc p) c -> p kc c", p=128)  # [128,32,256]

    with tile.TileContext(nc) as tc:
        with (
            tc.tile_pool(name="const", bufs=1) as constp,
            tc.tile_pool(name="stage", bufs=3) as stagep,
            tc.tile_pool(name="work", bufs=3) as workp,
            tc.tile_pool(name="expT", bufs=2) as expTp,
            tc.tile_pool(name="qg", bufs=2) as qgp,
            tc.tile_pool(name="og", bufs=2) as ogp,
            tc.tile_pool(name="small", bufs=8) as smallp,
            tc.tile_pool(name="psA", bufs=4, space="PSUM") as psmm,
            tc.tile_pool(name="psB", bufs=4, space="PSUM") as pstr,
        ):
            ident_sb = constp.tile([128, 128], f32, tag="ident")
            nc.sync.dma_start(ident_sb, ident_d[:, :])

            # ---- stream + round the DMA-fed matmul operands to f32r ----
            wq_sb = constp.tile([128, 2, INNER], f32r, tag="wq")
            for cc in range(2):
                t = stagep.tile([128, 512], f32, tag="wstage", name=f"wq{cc}")
                nc.sync.dma_start(t, wq_r_d[:, cc, :])
                nc.gpsimd.tensor_copy(wq_sb[:, cc, :], t)
            wkv_sb = constp.tile([128, 2, 2 * INNER], f32r, tag="wkv")
            for cc in range(2):
                for hh in range(2):
                    t = stagep.tile([128, 512], f32, tag="wstage", name=f"wkv{cc}{hh}")
                    nc.sync.dma_start(t, wkv_r_d[:, cc, hh * 512:(hh + 1) * 512])
                    nc.gpsimd.tensor_copy(wkv_sb[:, cc, hh * 512:(hh + 1) * 512], t)
            wo_sb = constp.tile([128, 4, DIM], f32r, tag="wo")
            for oc in range(4):
                t = stagep.tile([128, 256], f32, tag="wstage", name=f"wo{oc}")
                nc.sync.dma_start(t, wo_r_d[:, oc, :])
                nc.gpsimd.tensor_copy(wo_sb[:, oc, :], t)
            x_sb = constp.tile([128, 2, NPOS], f32r, tag="x")
            for cc in range(2):
                for qq in range(4):
                    t = stagep.tile([128, 1024], f32, tag="xstage", name=f"x{cc}{qq}")
                    nc.sync.dma_start(t, xs_r[:, cc, qq * 1024:(qq + 1) * 1024])
                    nc.gpsimd.tensor_copy(x_sb[:, cc, qq * 1024:(qq + 1) * 1024], t)

            xred_sb = constp.tile([128, 2, NKEY], f32r, tag="xred")
            k_sb = constp.tile([128, 4, NKEY], f32r, tag="ksb")
            v_sb = constp.tile([128, 4, NKEY], f32, tag="vsb")
            vT_sb = constp.tile([128, HEADS, 4, DIM_HEAD], f32r, tag="vT")

            # ---- stage A: x_red[c, key] (two 128-channel chunks, K=4096) ----
            psA = [
                psmm.tile([128, NKEY], f32, tag="mm", name=f"psA{_mc}")
                for _mc in range(2)
            ]
            for kc in range(32):
                m3f = stagep.tile([128, NKEY], f32, tag="m3f")
                nc.sync.dma_start(m3f, m3cT_d[kc * 128:(kc + 1) * 128, :])
                m3_t = stagep.tile([128, NKEY], f32r, tag="m3r")
                nc.gpsimd.tensor_copy(m3_t, m3f)
                xnf = stagep.tile([128, 256], f32, tag="xnf")
                nc.sync.dma_start(xnf, xn_r_d[:, kc, :])
                xn_t = stagep.tile([128, 256], f32r, tag="xnr")
                nc.gpsimd.tensor_copy(xn_t, xnf)
                for mc in range(2):
                    nc.tensor.matmul(
                        psA[mc], xn_t[:, mc * 128:(mc + 1) * 128], m3_t,
                        start=(kc == 0), stop=(kc == 31), skip_group_check=True,
                    )
            for mc in range(2):
                nc.vector.tensor_copy(xred_sb[:, mc, :], psA[mc])

            # ---- stage B: k,v projections (head-major channels) ----
            for oc in range(8):
                ps = psmm.tile([128, NKEY], f32, tag="mm")
                for cc in range(2):
                    nc.tensor.matmul(
                        ps, wkv_sb[:, cc, oc * 128:(oc + 1) * 128], xred_sb[:, cc, :],
                        start=(cc == 0), stop=(cc == 1),
                    )
                if oc < 4:
                    nc.vector.tensor_copy(k_sb[:, oc, :], ps)
                else:
                    nc.vector.tensor_copy(v_sb[:, oc - 4, :], ps)

            # ---- stage C: vT per head (PE transposes of v_h) ----
            for h in range(HEADS):
                p0 = (h % 2) * 64
                for jc in range(4):
                    pst = pstr.tile([128, 128], f32, tag="tr")
                    nc.tensor.transpose(
                        pst[:, :64],
                        v_sb[p0:p0 + 64, h // 2, jc * 128:(jc + 1) * 128],
                        ident_sb[p0:p0 + 64, p0:p0 + 64],
                    )
                    nc.vector.tensor_copy(vT_sb[:, h, jc, :], pst[:, :64])

            # ---- main loop: q projection + attention, per 512-position group ----
            for g in range(8):
                q_g = qgp.tile([128, 4, 512], f32r, tag="qg")
                for mc in range(4):
                    ps = psmm.tile([128, 512], f32, tag="mm")
                    for cc in range(2):
                        nc.tensor.matmul(
                            ps, wq_sb[:, cc, mc * 128:(mc + 1) * 128],
                            x_sb[:, cc, g * 512:(g + 1) * 512],
                            start=(cc == 0), stop=(cc == 1),
                        )
                    if mc % 2 == 0:
                        nc.scalar.copy(q_g[:, mc, :], ps)
                    else:
                        nc.vector.tensor_copy(q_g[:, mc, :], ps)

                o_g = ogp.tile([128, 4, 512], f32r, tag="og")
                for h in range(HEADS):
                    p0 = (h % 2) * 64
                    mcq = h // 2
                    expT_t = expTp.tile([128, 4, 512], f32r, tag="expT")
                    for pc in range(4):
                        c = g * 4 + pc
                        ps_a = psmm.tile([128, NKEY], f32, tag="mm")
                        nc.tensor.matmul(
                            ps_a,
                            q_g[p0:p0 + 64, mcq, pc * 128:(pc + 1) * 128],
                            k_sb[p0:p0 + 64, mcq, :],
                            start=True, stop=True,
                        )
                        exp_t = workp.tile([128, NKEY], f32, tag="exp")
                        sum_t = smallp.tile([128, 1], f32, tag="sum")
                        nc.scalar.activation(exp_t, ps_a, Exp, scale=SCALE, accum_out=sum_t)
                        rec_t = smallp.tile([128, 1], f32, tag="rec")
                        nc.vector.reciprocal(rec_t, sum_t)
                        attn_t = workp.tile([128, NKEY], f32, tag="attn")
                        nc.vector.tensor_scalar_mul(attn_t, exp_t, rec_t)
                        nc.sync.dma_start(attn_d[h, c * 128:(c + 1) * 128, :], attn_t)
                        for jc in range(4):
                            pst = pstr.tile([128, 128], f32, tag="tr")
                            nc.tensor.transpose(
                                pst, attn_t[:, jc * 128:(jc + 1) * 128], ident_sb
                            )
                            if jc % 2 == 0:
                                nc.vector.tensor_copy(
                                    expT_t[:, jc, pc * 128:(pc + 1) * 128], pst
                                )
                            else:
                                nc.scalar.copy(
                                    expT_t[:, jc, pc * 128:(pc + 1) * 128], pst
                                )
                    ps_o = psmm.tile([64, 512], f32, tag="mm")
                    for jc in range(4):
                        nc.tensor.matmul(
                            ps_o, vT_sb[:, h, jc, :], expT_t[:, jc, :],
                            start=(jc == 0), stop=(jc == 3),
                        )
                    if h % 2 == 0:
                        nc.vector.tensor_copy(o_g[p0:p0 + 64, mcq, :], ps_o)
                    else:
                        nc.scalar.copy(o_g[p0:p0 + 64, mcq, :], ps_o)

                # ---- out projection for this pos-group ----
                for mc2 in range(2):
                    ps_f = psmm.tile([128, 512], f32, tag="mm")
                    for oc in range(4):
                        nc.tensor.matmul(
                            ps_f, wo_sb[:, oc, mc2 * 128:(mc2 + 1) * 128], o_g[:, oc, :],
                            start=(oc == 0), stop=(oc == 3),
                        )
                    out_t = workp.tile([128, 512], f32, tag="outt")
                    nc.vector.tensor_copy(out_t, ps_f)
                    nc.sync.dma_start(out_r[:, mc2, g * 512:(g + 1) * 512], out_t)

    nc.finalize()
    return nc


def kernel(x, w_qkv, w_out):
    from concourse.bass_utils import run_bass_kernel_spmd

    x = np.asarray(x, dtype=_F32)
    w_qkv = np.asarray(w_qkv, dtype=_F32)
    w_out = np.asarray(w_out, dtype=_F32)

    idx, M3cT, hm = _host_constants()

    xf = x[0]                                             # [256, 32, 32, 32]
    xneed = xf[:, idx][:, :, idx][:, :, :, idx].reshape(DIM, NEED ** 3)
    xneedT = np.ascontiguousarray(xneed.T)                # [4096, 256]

    Wq = w_qkv[0:INNER][hm]                               # head-major rows
    Wk = w_qkv[INNER:2 * INNER][hm]
    Wv = w_qkv[2 * INNER:3 * INNER][hm]
    wqT = np.ascontiguousarray(Wq.T)                      # [256, 512]
    wkvT = np.ascontiguousarray(np.concatenate([Wk, Wv], 0).T)   # [256, 1024]
    woT = np.ascontiguousarray(w_out[:, hm].T)            # [512, 256]
    ident = np.eye(128, dtype=_F32)

    if "nc" not in _NC_CACHE:
        _NC_CACHE["nc"] = _build_nc()
    nc = _NC_CACHE["nc"]

    shared = {
        "xneedT": xneedT, "m3cT": M3cT, "wqT": wqT, "wkvT": wkvT,
        "woT": woT, "ident": ident,
    }
    in_maps = []
    for i in range(NCORES):
        xs = np.ascontiguousarray(
            xf[:, i * DSL:(i + 1) * DSL].reshape(DIM, NPOS)
        )
        in_maps.append({"xs": xs, **shared})

    import os
    trace = bool(int(os.environ.get("KERNEL_TRACE", "0")))
    res = run_bass_kernel_spmd(
        nc, in_maps, core_ids=list(range(NCORES)), trace=trace
    )
    _NC_CACHE["last_result"] = res
    results = res.results

    out = np.concatenate(
        [r["out_s"].reshape(DIM, DSL, H, W) for r in results], axis=1
    )[None]                                               # [1, 256, 32, 32, 32]
    attn = np.concatenate([r["attn_s"] for r in results], axis=1)[None]
    return out, attn


# revision 20
# speedup vs baseline: 1.4138x; 1.1942x over previous
"""Trainium2 Bass kernel for nn_LinearAttention (3D linear-attention block).

Math (faithful to the reference, but algebraically restructured):
  qkv = w_qkv @ x              (pointwise conv, x: [256, 32^3])
  k,v are trilinearly downsampled to 8^3 BEFORE the projection
      (downsample commutes with the channel projection -> huge FLOP saving)
  per head h (8 heads, dim_head 64, channel c = d*8 + h):
      attn = softmax(q_h^T k_h * 64^-0.5)   [N=32768, M=512]
      o_h = attn @ v_h
  out = w_out @ o

Sharding: split the depth axis D=32 into 8 slices of 4 planes -> each core
handles 4096 query positions (all heads). k/v (512 keys) are computed
redundantly on every core from a small replicated subset of x (the trilinear
interp with align_corners=True only reads 16 of 32 planes per axis).

Performance notes:
  - fp32 matmuls lower to TWO hardware passes on TRN2; float32r (rounded
    fp32, ~tf32 precision: measured 1.5e-4 rel err) runs single-pass at
    ~227ns per 512-col matmul.  All matmuls take float32r operands; the
    rounding "producer" ops walrus requires are folded into the PSUM->SBUF
    evacuation copies where possible, and done on the otherwise-idle GpSimd
    engine for DMA-fed operands.
  - exp(scale*logits) runs on ScalarE straight out of PSUM with accumulated
    row sums (accum_out), softmax normalize on VectorE.
  - o = attn @ v needs attn^T; done via PE transposes of the normalized
    attn tiles (fp32, single pass), evacuated to f32r SBUF tiles split
    between ScalarE and VectorE.
"""

import numpy as np

HEADS = 8
DIM_HEAD = 64
INNER = 512
DIM = 256
R = 8
SCALE = DIM_HEAD ** -0.5
D = H = W = 32
N = D * H * W          # 32768
NCORES = 8
DSL = D // NCORES      # 4 depth planes per core
NPOS = DSL * H * W     # 4096 query positions per core
NKEY = R * R * R       # 512 keys
NEED = 16              # input grid points read by the 32->8 interp, per axis

_F32 = np.float32


def _interp_matrix(n_in, n_out):
    # replicate reference.interp_matrix (f32, align_corners=True)
    pos = np.arange(n_out, dtype=_F32) * _F32((n_in - 1) / (n_out - 1))
    lo = np.clip(np.floor(pos).astype(np.int32), 0, n_in - 2)
    frac = pos - lo.astype(_F32)
    M = np.zeros((n_out, n_in), _F32)
    rows = np.arange(n_out)
    M[rows, lo] += _F32(1.0) - frac
    M[rows, lo + 1] += frac
    return M, lo


def _host_constants():
    M, lo = _interp_matrix(D, R)
    idx = np.unique(np.concatenate([lo, lo + 1]))          # 16 needed planes
    assert len(idx) == NEED
    Mc = np.ascontiguousarray(M[:, idx])                   # [8, 16]
    # compact 3D interp operator [512 keys, 4096 needed points]
    M3c = np.einsum("ai,bj,ck->abcijk", Mc, Mc, Mc).reshape(NKEY, NEED ** 3)
    M3cT = np.ascontiguousarray(M3c.T.astype(_F32))        # [4096, 512]
    # head-major channel permutation: hm[h*64+d] = d*8+h
    hm = (np.arange(HEADS)[:, None] + HEADS * np.arange(DIM_HEAD)[None, :]).reshape(-1)
    return idx, M3cT, hm


_NC_CACHE = {}


def _build_nc():
    import concourse.bacc as bacc
    import concourse.bass as bass
    import concourse.tile as tile
    from concourse import mybir

    f32 = mybir.dt.float32
    f32r = mybir.dt.float32r
    Exp = mybir.ActivationFunctionType.Exp

    nc = bacc.Bacc("TRN2", target_bir_lowering=False, debug=False)

    xs_d = nc.dram_tensor("xs", [DIM, NPOS], f32, kind="ExternalInput")
    xneedT_d = nc.dram_tensor("xneedT", [NEED ** 3, DIM], f32, kind="ExternalInput")
    m3cT_d = nc.dram_tensor("m3cT", [NEED ** 3, NKEY], f32, kind="ExternalInput")
    wqT_d = nc.dram_tensor("wqT", [DIM, INNER], f32, kind="ExternalInput")
    wkvT_d = nc.dram_tensor("wkvT", [DIM, 2 * INNER], f32, kind="ExternalInput")
    woT_d = nc.dram_tensor("woT", [INNER, DIM], f32, kind="ExternalInput")
    ident_d = nc.dram_tensor("ident", [128, 128], f32, kind="ExternalInput")

    out_d = nc.dram_tensor("out_s", [DIM, NPOS], f32, kind="ExternalOutput")
    attn_d = nc.dram_tensor("attn_s", [HEADS, NPOS, NKEY], f32, kind="ExternalOutput")

    out_r = out_d[:, :].rearrange("(mc p) n -> p mc n", p=128)   # [128, 2, 4096]
    xs_r = xs_d[:, :].rearrange("(c p) n -> p c n", p=128)       # [128, 2, 4096]
    wq_r_d = wqT_d[:, :].rearrange("(c p) m -> p c m", p=128)    # [128, 2, 512]
    wkv_r_d = wkvT_d[:, :].rearrange("(c p) m -> p c m", p=128)  # [128, 2, 1024]
    wo_r_d = woT_d[:, :].rearrange("(c p) m -> p c m", p=128)    # [128, 4, 256]
    xn_r_d = xneedT_d[:, :].rearrange("(kc p) c -> p kc c", p=128)  # [128,32,256]

    with tile.TileContext(nc) as tc:
        with (
            tc.tile_pool(name="const", bufs=1) as constp,
            tc.tile_pool(name="stage", bufs=3) as stagep,
            tc.tile_pool(name="work", bufs=3) as workp,
            tc.tile_pool(name="expT", bufs=2) as expTp,
            tc.tile_pool(name="qg", bufs=2) as qgp,
            tc.tile_pool(name="og", bufs=2) as ogp,
            tc.tile_pool(name="small", bufs=8) as smallp,
            tc.tile_pool(name="psA", bufs=4, space="PSUM") as psmm,
            tc.tile_pool(name="psB", bufs=4, space="PSUM") as pstr,
            tc.tile_pool(name="dramp", bufs=4, space="DRAM") as dramp,
        ):
            ident_sb = constp.tile([128, 128], f32, tag="ident")
            nc.sync.dma_start(ident_sb, ident_d[:, :])

            # ---- stream + round the DMA-fed matmul operands to f32r ----
            wq_sb = constp.tile([128, 2, INNER], f32r, tag="wq")
            for cc in range(2):
                t = stagep.tile([128, 512], f32, tag="wstage", name=f"wq{cc}")
                nc.sync.dma_start(t, wq_r_d[:, cc, :])
                nc.gpsimd.tensor_copy(wq_sb[:, cc, :], t)
            wkv_sb = constp.tile([128, 2, 2 * INNER], f32r, tag="wkv")
            for cc in range(2):
                for hh in range(2):
                    t = stagep.tile([128, 512], f32, tag="wstage", name=f"wkv{cc}{hh}")
                    nc.sync.dma_start(t, wkv_r_d[:, cc, hh * 512:(hh + 1) * 512])
                    nc.gpsimd.tensor_copy(wkv_sb[:, cc, hh * 512:(hh + 1) * 512], t)
            wo_sb = constp.tile([128, 4, DIM], f32r, tag="wo")
            for oc in range(4):
                t = stagep.tile([128, 256], f32, tag="wstage", name=f"wo{oc}")
                nc.sync.dma_start(t, wo_r_d[:, oc, :])
                nc.gpsimd.tensor_copy(wo_sb[:, oc, :], t)
            x_sb = constp.tile([128, 2, NPOS], f32r, tag="x")
            for cc in range(2):
                for qq in range(4):
                    t = stagep.tile([128, 1024], f32, tag="xstage", name=f"x{cc}{qq}")
                    nc.sync.dma_start(t, xs_r[:, cc, qq * 1024:(qq + 1) * 1024])
                    nc.vector.tensor_copy(x_sb[:, cc, qq * 1024:(qq + 1) * 1024], t)

            xred_sb = constp.tile([128, 2, NKEY], f32r, tag="xred")
            k_sb = constp.tile([128, 4, NKEY], f32r, tag="ksb")
            v_sb = constp.tile([128, 4, NKEY], f32, tag="vsb")
            vT_sb = constp.tile([128, HEADS, 4, DIM_HEAD], f32r, tag="vT")

            # ---- stage A: x_red[c, key] (two 128-channel chunks, K=4096) ----
            psA = [
                psmm.tile([128, NKEY], f32, tag="mm", name=f"psA{_mc}")
                for _mc in range(2)
            ]
            for kc in range(32):
                m3f = stagep.tile([128, NKEY], f32, tag="m3f")
                nc.sync.dma_start(m3f, m3cT_d[kc * 128:(kc + 1) * 128, :])
                m3_t = stagep.tile([128, NKEY], f32r, tag="m3r")
                nc.vector.tensor_copy(m3_t, m3f)
                xnf = stagep.tile([128, 256], f32, tag="xnf")
                nc.sync.dma_start(xnf, xn_r_d[:, kc, :])
                xn_t = stagep.tile([128, 256], f32r, tag="xnr")
                nc.vector.tensor_copy(xn_t, xnf)
                for mc in range(2):
                    nc.tensor.matmul(
                        psA[mc], xn_t[:, mc * 128:(mc + 1) * 128], m3_t,
                        start=(kc == 0), stop=(kc == 31), skip_group_check=True,
                    )
            for mc in range(2):
                nc.vector.tensor_copy(xred_sb[:, mc, :], psA[mc])

            # ---- stage B: k,v projections (head-major channels) ----
            for oc in range(8):
                ps = psmm.tile([128, NKEY], f32, tag="mm")
                for cc in range(2):
                    nc.tensor.matmul(
                        ps, wkv_sb[:, cc, oc * 128:(oc + 1) * 128], xred_sb[:, cc, :],
                        start=(cc == 0), stop=(cc == 1),
                    )
                if oc < 4:
                    nc.vector.tensor_copy(k_sb[:, oc, :], ps)
                else:
                    nc.vector.tensor_copy(v_sb[:, oc - 4, :], ps)

            # ---- stage C: vT per head (PE transposes of v_h) ----
            for h in range(HEADS):
                p0 = (h % 2) * 64
                for jc in range(4):
                    pst = pstr.tile([128, 128], f32, tag="tr")
                    nc.tensor.transpose(
                        pst[:, :64],
                        v_sb[p0:p0 + 64, h // 2, jc * 128:(jc + 1) * 128],
                        ident_sb[p0:p0 + 64, p0:p0 + 64],
                    )
                    nc.vector.tensor_copy(vT_sb[:, h, jc, :], pst[:, :64])

            # ---- main loop: q projection + attention, per 512-position group ----
            for g in range(8):
                q_g = qgp.tile([128, 4, 512], f32r, tag="qg")
                for mc in range(4):
                    ps = psmm.tile([128, 512], f32, tag="mm")
                    for cc in range(2):
                        nc.tensor.matmul(
                            ps, wq_sb[:, cc, mc * 128:(mc + 1) * 128],
                            x_sb[:, cc, g * 512:(g + 1) * 512],
                            start=(cc == 0), stop=(cc == 1),
                        )
                    nc.vector.tensor_copy(q_g[:, mc, :], ps)

                o_g = ogp.tile([128, 4, 512], f32r, tag="og")
                for h in range(HEADS):
                    p0 = (h % 2) * 64
                    mcq = h // 2
                    # pos-major logits -> exp (+row sums) -> normalized attn out
                    rec_g = smallp.tile([128, 4], f32, tag="recg")
                    for pc in range(4):
                        c = g * 4 + pc
                        ps_a = psmm.tile([128, NKEY], f32, tag="mm")
                        nc.tensor.matmul(
                            ps_a,
                            q_g[p0:p0 + 64, mcq, pc * 128:(pc + 1) * 128],
                            k_sb[p0:p0 + 64, mcq, :],
                            start=True, stop=True,
                        )
                        exp_t = workp.tile([128, NKEY], f32, tag="exp")
                        sum_t = smallp.tile([128, 1], f32, tag="sum")
                        nc.scalar.activation(exp_t, ps_a, Exp, scale=SCALE, accum_out=sum_t)
                        nc.vector.reciprocal(rec_g[:, pc:pc + 1], sum_t)
                        attn_t = workp.tile([128, NKEY], f32, tag="attn")
                        nc.vector.tensor_scalar_mul(attn_t, exp_t, rec_g[:, pc:pc + 1])
                        nc.sync.dma_start(attn_d[h, c * 128:(c + 1) * 128, :], attn_t)

                    # key-major logits -> exp -> f32r expT (no copies, no transposes)
                    expT_t = expTp.tile([128, 4, 512], f32r, tag="expT")
                    for jc in range(4):
                        ps_at = psmm.tile([128, NKEY], f32, tag="mm")
                        nc.tensor.matmul(
                            ps_at,
                            k_sb[p0:p0 + 64, mcq, jc * 128:(jc + 1) * 128],
                            q_g[p0:p0 + 64, mcq, :],
                            start=True, stop=True,
                        )
                        nc.scalar.activation(expT_t[:, jc, :], ps_at, Exp, scale=SCALE)

                    # recip row: [128,4] -> PE transpose -> [4,128] -> [1,512]
                    # -> broadcast to [64,512] (DMA partition-replication)
                    ps_r = pstr.tile([4, 128], f32, tag="tr")
                    nc.tensor.transpose(ps_r, rec_g, ident_sb)
                    rT_t = smallp.tile([4, 128], f32, tag="rT")
                    nc.vector.tensor_copy(rT_t, ps_r)
                    rdram = dramp.tile([512], f32, tag="rdram")
                    nc.sync.dma_start(rdram, rT_t)
                    rrow_b = smallp.tile([64, 512], f32, tag="rrowb", bufs=3)
                    nc.sync.dma_start(
                        rrow_b,
                        bass.AP(
                            tensor=rdram.tensor, offset=rdram.offset,
                            ap=[[0, 64], [1, 512]],
                        ),
                    )

                    ps_o = psmm.tile([64, 512], f32, tag="mm")
                    for jc in range(4):
                        nc.tensor.matmul(
                            ps_o, vT_sb[:, h, jc, :], expT_t[:, jc, :],
                            start=(jc == 0), stop=(jc == 3),
                        )
                    nc.vector.tensor_tensor(
                        o_g[p0:p0 + 64, mcq, :], ps_o, rrow_b,
                        mybir.AluOpType.mult,
                    )

                # ---- out projection for this pos-group ----
                for mc2 in range(2):
                    ps_f = psmm.tile([128, 512], f32, tag="mm")
                    for oc in range(4):
                        nc.tensor.matmul(
                            ps_f, wo_sb[:, oc, mc2 * 128:(mc2 + 1) * 128], o_g[:, oc, :],
                            start=(oc == 0), stop=(oc == 3),
                        )
                    out_t = workp.tile([128, 512], f32, tag="outt")
                    nc.vector.tensor_copy(out_t, ps_f)
                    nc.sync.dma_start(out_r[:, mc2, g * 512:(g + 1) * 512], out_t)

    nc.finalize()
    return nc


def kernel(x, w_qkv, w_out):
    from concourse.bass_utils import run_bass_kernel_spmd

    x = np.asarray(x, dtype=_F32)
    w_qkv = np.asarray(w_qkv, dtype=_F32)
    w_out = np.asarray(w_out, dtype=_F32)

    idx, M3cT, hm = _host_constants()

    xf = x[0]                                             # [256, 32, 32, 32]
    xneed = xf[:, idx][:, :, idx][:, :, :, idx].reshape(DIM, NEED ** 3)
    xneedT = np.ascontiguousarray(xneed.T)                # [4096, 256]

    Wq = w_qkv[0:INNER][hm]                               # head-major rows
    Wk = w_qkv[INNER:2 * INNER][hm]
    Wv = w_qkv[2 * INNER:3 * INNER][hm]
    wqT = np.ascontiguousarray(Wq.T)                      # [256, 512]
    wkvT = np.ascontiguousarray(np.concatenate([Wk, Wv], 0).T)   # [256, 1024]
    woT = np.ascontiguousarray(w_out[:, hm].T)            # [512, 256]
    ident = np.eye(128, dtype=_F32)

    if "nc" not in _NC_CACHE:
        _NC_CACHE["nc"] = _build_nc()
    nc = _NC_CACHE["nc"]

    shared = {
        "xneedT": xneedT, "m3cT": M3cT, "wqT": wqT, "wkvT": wkvT,
        "woT": woT, "ident": ident,
    }
    in_maps = []
    for i in range(NCORES):
        xs = np.ascontiguousarray(
            xf[:, i * DSL:(i + 1) * DSL].reshape(DIM, NPOS)
        )
        in_maps.append({"xs": xs, **shared})

    import os
    trace = bool(int(os.environ.get("KERNEL_TRACE", "0")))
    res = run_bass_kernel_spmd(
        nc, in_maps, core_ids=list(range(NCORES)), trace=trace
    )
    _NC_CACHE["last_result"] = res
    results = res.results

    out = np.concatenate(
        [r["out_s"].reshape(DIM, DSL, H, W) for r in results], axis=1
    )[None]                                               # [1, 256, 32, 32, 32]
    attn = np.concatenate([r["attn_s"] for r in results], axis=1)[None]
    return out, attn
